# revision 1
# baseline (speedup 1.0000x reference)
"""Trainium2 Bass kernel for nn_AutoregressiveConvLSTM — fp8 DoubleRow version.

Data-parallel over batch: 32 images -> 8 cores x 4 images; inside a core the
4 images are split into two pairs (A, B) that are software-pipelined so the
TensorEngine never starves (keeps the p-state ramp at full speed).

All convs run as DoubleRow fp8 matmuls: each instruction contracts TWO
(tridiagonal-band, plane) products at once at 0.5 cycles/row.  The rhs
[K, 2, N] pair dim is a hand-built AP stride, so any two planes of
Z = [zi, h0, h1, ones] x (BP imgs) x 131 cols can pair, including the
overlapping (zi@dx0, zi@dx1) pair and (h1@2, ones@0) which carries the
per-gate bias as a diagonal band.  5 DR per gate half, 3 per conv_out
channel, 2 for conv_in.

Cell math in bf16 on DVE (fast 2x/4x modes where eligible), z/z^2/acc on
the Pool engine, tanh/exp on Act.  conv_out for step t is evaluated at step
t+1 (its h input is long since ready) so the PE sequencer never blocks and
the p-state ramp stays at 2.4 GHz.

sigmoid(v) = (tanh(v/2)+1)/2; h is stored doubled (h2 = 2h) with the 0.5
folded into the Whh/Wout bands; the 1/2 gate scale is folded into the i,f,o
bands so every gate tanh runs with scale=1, bias=0; cell state kept as
S2 = 2c with tanh(c) = tanh(0.5*S2).
"""

import sys
import numpy as np
import ml_dtypes

for _p in ("/opt/trn_rl_repo", "/root/.axon_site/_ro/trn_rl_repo"):
    if _p not in sys.path:
        sys.path.insert(0, _p)

import concourse.bacc as bacc
import concourse.mybir as mybir
from concourse import bass, tile
from concourse.bass_utils import run_bass_kernel_spmd

F32 = mybir.dt.float32
BF16 = mybir.dt.bfloat16
F8 = mybir.dt.float8e4
AF = mybir.ActivationFunctionType
ALU = mybir.AluOpType
DR = mybir.MatmulPerfMode.DoubleRow
FP8 = ml_dtypes.float8_e4m3

B, C, H, W = 32, 16, 128, 128
NCORES = 8
BL = B // NCORES          # images per core = 4
BP = 2                    # images per pair
WP = 131                  # padded row width (needs +1 headroom for zis@dx2)
T = C - 1                 # 15 recurrence steps
LOG2PI = 1.8378770664093453

# Z slot indices
S_ZI, S_H0, S_H1, S_ONE = 0, 1, 2, 3


def _band(w_col):
    """128x128 tridiagonal lhsT: band[k, m] = w_col[k - m + 1]."""
    Bm = np.zeros((H, H), np.float32)
    idx = np.arange(H)
    for ky in range(3):
        hh = idx + ky - 1
        m = (hh >= 0) & (hh < H)
        Bm[hh[m], idx[m]] = w_col[ky]
    return Bm


def _build_band_pairs(Win, Wih, Whh, Wout, gb):
    """Band pair tiles [N, 2, H, H] fp32 (quantized to fp8 at upload) plus
    index maps.  Gate co order: i0,i1,g0,g1,f0,f1,o0,o1 (co = gate*2 + feat).

    Scales: i,f,o gates folded x0.5 (tanh(v/2)); h inputs folded x0.5 (h2=2h).
    """
    pairs = []

    def add(a, b):
        pairs.append(np.stack([a, b]))
        return len(pairs) - 1

    Z128 = np.zeros((H, H), np.float32)

    # conv_in (x -> zi): pairs (x@0,x@1), (x@2,junk)
    zi_idx = [
        add(_band(Win[:, 0, 0, 0]), _band(Win[:, 1, 0, 0])),
        add(_band(Win[:, 2, 0, 0]), Z128),
    ]

    # gate halves: 5 pairs covering (zi@0,zi@1),(zi@2,h0@0),(h0@1,h0@2),
    # (h1@0,h1@1),(h1@2,ones@0=bias diag)
    gate_idx = {}
    for co in range(8):
        g = co // 2
        sg = 1.0 if g == 1 else 0.5          # gate scale (g gate unscaled)
        sh = 0.5 * sg                        # h2 folding on Whh
        bv = sg * (gb[co] + (1.0 if g == 2 else 0.0))   # forget-gate +1
        gate_idx[co] = [
            add(_band(sg * Wih[:, 0, 0, co]), _band(sg * Wih[:, 1, 0, co])),
            add(_band(sg * Wih[:, 2, 0, co]), _band(sh * Whh[:, 0, 0, co])),
            add(_band(sh * Whh[:, 1, 0, co]), _band(sh * Whh[:, 2, 0, co])),
            add(_band(sh * Whh[:, 0, 1, co]), _band(sh * Whh[:, 1, 1, co])),
            add(_band(sh * Whh[:, 2, 1, co]),
                np.diag(np.full(H, bv, np.float32))),
        ]

    co_idx = {}        # out co (0=mu, 1=ls) -> [3 pair indices]
    Wout_y = Wout[:, :, :2, :]
    for co in range(2):
        co_idx[co] = [
            add(_band(0.5 * Wout_y[:, dx, 0, co]), _band(0.5 * Wout_y[:, dx, 1, co]))
            for dx in range(3)
        ]

    return np.stack(pairs), zi_idx, gate_idx, co_idx


_CACHED = {}


def _build_program():
    import os
    nc = bacc.Bacc(None, target_bir_lowering=False)

    NBANDS = 2 + 40 + 6
    xq_d = nc.dram_tensor("xq", [H, C * BL * WP], F8, kind="ExternalInput")
    xh_d = nc.dram_tensor("xh", [H, C * BL * W], BF16, kind="ExternalInput")
    bp_d = nc.dram_tensor("bp", [H, NBANDS * 2 * H], F8, kind="ExternalInput")
    sc_d = nc.dram_tensor("sc", [H, 8], F32, kind="ExternalInput")
    out_d = nc.dram_tensor("out", [BL, 1], F32, kind="ExternalOutput")

    TR = int(os.environ.get("KERNEL_T", T))

    GP = int(os.environ.get("GP_BUFS", 2))
    CP = int(os.environ.get("CP_BUFS", 2))
    ZP = int(os.environ.get("ZP_BUFS", 2))
    with tile.TileContext(nc) as tc:
        with (
            tc.tile_pool(name="const", bufs=1) as cpool,
            tc.tile_pool(name="state", bufs=1) as spool,
            tc.tile_pool(name="gp", bufs=GP, space=bass.MemorySpace.PSUM) as gpool,
            tc.tile_pool(name="cp", bufs=CP, space=bass.MemorySpace.PSUM) as copool,
            tc.tile_pool(name="zp", bufs=ZP, space=bass.MemorySpace.PSUM) as zpool,
        ):
            xq = cpool.tile([H, C, BL, WP], F8, tag="xq")
            xh = cpool.tile([H, C, BL, W], BF16, tag="xh")
            bp = cpool.tile([H, NBANDS, 2, H], F8, tag="bp")
            sc = cpool.tile([H, 8], F32, tag="sc")
            ones = cpool.tile([H, 1], F32, tag="ones")

            Z = spool.tile([H, 2, 4, BP, WP], F8, tag="Z")
            th2 = spool.tile([H, 2, 2, 8, BP, W], BF16, tag="th")
            S2 = spool.tile([H, 2, 2, 2, BP, W], BF16, tag="S2")
            tcn2 = spool.tile([H, 2, 2, 2, BP, W], BF16, tag="tcn")
            Et2 = spool.tile([H, 2, 2, BP, W], BF16, tag="Et")
            dt2 = spool.tile([H, 2, 2, BP, W], BF16, tag="dt")
            zt2 = spool.tile([H, 2, 2, BP, W], BF16, tag="zt")
            zq2 = spool.tile([H, 2, 2, BP, W], F32, tag="zq")
            acc = spool.tile([H, BL, W], F32, tag="acc")
            lscols = spool.tile([H, BL, T], F32, tag="lscols")

            # ---- loads ----
            if os.environ.get("DMAV", "0") == "1":
                nc.sync.dma_start(sc[:], sc_d[:])
                nc.sync.dma_start(bp[:, 0:2], bp_d[:, 0:2 * 2 * H])
                nc.sync.dma_start(xq[:, 0:4], xq_d[:, 0:4 * BL * WP])
                nc.sync.dma_start(xh[:, 0:8], xh_d[:, 0:8 * BL * W])
                nc.sync.dma_start(bp[:, 2:], bp_d[:, 2 * 2 * H:])
                nc.sync.dma_start(xq[:, 4:], xq_d[:, 4 * BL * WP:])
                nc.sync.dma_start(xh[:, 8:], xh_d[:, 8 * BL * W:])
            else:
                nc.sync.dma_start(bp[:], bp_d[:])
                nc.sync.dma_start(sc[:], sc_d[:])
                nc.sync.dma_start(xh[:, 0:4], xh_d[:, 0:4 * BL * W])
                nc.sync.dma_start(xq[:, 0:4], xq_d[:, 0:4 * BL * WP])
                nc.sync.dma_start(xq[:, 4:], xq_d[:, 4 * BL * WP:])
                nc.sync.dma_start(xh[:, 4:], xh_d[:, 4 * BL * W:])

            nc.gpsimd.memset(Z[:], 0.0)
            nc.gpsimd.memset(S2[:], 0.0)
            nc.gpsimd.memset(lscols[:], 0.0)
            nc.gpsimd.memset(ones[:], 1.0)
            # ones slots of Z
            for P in range(2):
                nc.gpsimd.memset(Z[:, P, S_ONE], 1.0)

            def band(i):
                return bp[:, i]

            def mm(out, i, rhs, start, stop):
                nc.tensor.matmul(out, band(i), rhs, start=start, stop=stop,
                                 perf_mode=DR, skip_group_check=True)

            # scale/bias columns (sc): [0]=e^{-b1}, [1]=-b0*e^{-b1} for ch0
            # ch0 logprob: acc = Square(x0 * e^{-b1} - b0 e^{-b1})
            nc.scalar.activation(acc[:], xh[:, 0], AF.Square,
                                 scale=sc[:, 0:1], bias=sc[:, 1:2])

            def _pair(base, delta):
                # insert a DoubleRow pair dim (stride delta, num 2) after the
                # partition dim of a [H, BP, W] plane AP
                return bass.AP(
                    base.tensor, base.offset,
                    [list(base.ap[0]), [delta, 2]]
                    + [list(d) for d in base.ap[1:]])

            def zpair(P, slot_a, dx_a, slot_b, dx_b):
                base = Z[:, P, slot_a, :, dx_a:dx_a + W]
                return _pair(base, (slot_b - slot_a) * BP * WP + dx_b - dx_a)

            # ---- zi conv for (t, P) -> psum tile ----
            def zi_mm(t, P):
                zp = zpool.tile([H, BP, W], F32, tag="zp")
                base0 = xq[:, t, P * BP:(P + 1) * BP, 0:W]
                base2 = xq[:, t, P * BP:(P + 1) * BP, 2:2 + W]
                mm(zp[:], ZI_IDX[0], _pair(base0, 1), True, False)
                mm(zp[:], ZI_IDX[1], _pair(base2, 1), False, True)
                return zp

            def zi_copy(zp, P):
                # Z.zi[1:129] <- zp + b_in  (PSUM -> DVE)
                nc.vector.tensor_scalar(
                    Z[:, P, S_ZI, :, 1:1 + W], zp[:], sc[:, 2:3], None,
                    op0=ALU.add)

            # ---- gate chunk: 4 halves; the pure-zi DR first (independent
            # of h2, keeps PE busy across the recurrence boundary), then
            # the h-tap DRs.  Interleaved accumulation groups per co-slot.
            def gate_chunk(gtile, P, cos):
                p_zi = zpair(P, S_ZI, 0, S_ZI, 1)
                p_zh = zpair(P, S_ZI, 2, S_H0, 0)
                p_h0 = zpair(P, S_H0, 1, S_H0, 2)
                p_h1 = zpair(P, S_H1, 0, S_H1, 1)
                p_hb = zpair(P, S_H1, 2, S_ONE, 0)
                for k, co in enumerate(cos):
                    mm(gtile[:, k], GATE_IDX[co][0], p_zi, True, False)
                for k, co in enumerate(cos):
                    idx = GATE_IDX[co]
                    out = gtile[:, k]
                    mm(out, idx[1], p_zh, False, False)
                    mm(out, idx[2], p_h0, False, False)
                    mm(out, idx[3], p_h1, False, False)
                    mm(out, idx[4], p_hb, False, True)

            # ---- conv_out for step t, pair P ----
            def co_mm(t, P):
                cp = copool.tile([H, 2, BP, W], F32, tag="cp")
                for co in range(2):
                    for dx in range(3):
                        h = zpair(P, S_H0, dx, S_H1, dx)
                        mm(cp[:, co], CO_IDX[co][dx], h, dx == 0, dx == 2)
                return cp

            # ---- logprob pieces for channel ch from conv_out psum cp ----
            def lp_exp(cp, P, pb):
                # E = exp(-(ls) - b1)
                nc.scalar.activation(Et2[:, pb, P], cp[:, 1], AF.Exp,
                                     scale=-1.0, bias=sc[:, 3:4])

            def lp_dls(cp, ch, P, tcol, pb):
                ps = slice(P * BP, (P + 1) * BP)
                # d = (mu + b0) - x_ch   (PSUM -> DVE)
                nc.vector.scalar_tensor_tensor(
                    dt2[:, pb, P], cp[:, 0], sc[:, 4:5], xh[:, ch, ps],
                    ALU.add, ALU.subtract)
                # per-(step,pair) ls column sums  (PSUM -> DVE)
                nc.vector.tensor_reduce(
                    lscols[:, ps, tcol:tcol + 1], cp[:, 1],
                    axis=mybir.AxisListType.X, op=ALU.add)

            def lp_pool(P, pb):
                ps = slice(P * BP, (P + 1) * BP)
                nc.gpsimd.tensor_tensor(zt2[:, pb, P], dt2[:, pb, P],
                                        Et2[:, pb, P], op=ALU.mult)
                nc.gpsimd.tensor_tensor(zq2[:, pb, P], zt2[:, pb, P],
                                        zt2[:, pb, P], op=ALU.mult)
                nc.gpsimd.tensor_tensor(acc[:, ps], acc[:, ps], zq2[:, pb, P],
                                        op=ALU.add)

            def logprob(cp, ch, P, tcol, pb):
                lp_exp(cp, P, pb)
                lp_dls(cp, ch, P, tcol, pb)
                lp_pool(P, pb)

            # temps for u1/u2
            u1t = spool.tile([H, 2, 2, BP, W], BF16, tag="u1t")
            u2t = spool.tile([H, 2, 2, BP, W], BF16, tag="u2t")
            jcol = spool.tile([H, 2], F32, tag="jcol")

            # ---- priming: zi(0) for both pairs ----
            zps = {}
            for P in range(2):
                zps[P] = zi_mm(0, P)
            for P in range(2):
                zi_copy(zps[P], P)

            def u1_emit(P, pp, th):
                # u1 = (tf*0.5 + 0.5) * S2prev  (0.5 of the cell pre-folded)
                nc.vector.affine_mul_reduce(
                    u1t[:, P].opt(), jcol[:, P:P + 1],
                    th[:, P, 0:2].opt(), S2[:, P, pp].opt(), 0.5, 0.5)

            def u2_s2(P, pc, f, th):
                # one cell feature: u2_f, S2_f
                nc.vector.scalar_tensor_tensor(
                    u2t[:, P, f], th[:, P, 4 + f], 1.0, th[:, P, 6 + f],
                    ALU.add, ALU.mult)
                nc.vector.tensor_tensor(
                    S2[:, P, pc, f], u1t[:, P, f], u2t[:, P, f], op=ALU.add)

            def tc_h2(P, pc, f, th, tcn):
                nc.scalar.activation(tcn[:, P, f], S2[:, P, pc, f], AF.Tanh,
                                     scale=0.5)
                nc.vector.scalar_tensor_tensor(
                    Z[:, P, S_H0 + f, :, 1:1 + W], th[:, P, 2 + f],
                    1.0, tcn[:, P, f], ALU.add, ALU.mult)

            for t in range(TR):
                pp, pc = (t + 1) % 2, t % 2
                th = th2[:, pc]
                tcn = tcn2[:, pc]
                gt = {}
                # PE: fo chunks both pairs
                for P in range(2):
                    g = gpool.tile([H, 4, BP, W], F32, tag="g")
                    gt[P] = g
                    gate_chunk(g, P, (4, 5, 6, 7))   # f0,f1,o0,o1
                # Act: tanh fo chunks -> th slots 0:4 (f0,f1,o0,o1)
                for P in range(2):
                    nc.scalar.activation(th[:, P, 0:4], gt[P][:], AF.Tanh)
                # DVE: u1 both pairs (only need tanh_fo + S2 prev)
                for P in range(2):
                    u1_emit(P, pp, th)
                # PE: coA(t-1); ig chunk A; ziA
                if t > 0:
                    cpA = co_mm(t - 1, 0)
                gate_chunk(gt[0], 0, (0, 1, 2, 3))   # i0,i1,g0,g1
                if t + 1 < TR:
                    zpA = zi_mm(t + 1, 0)
                # Act: tanh_igA ; expA(t-1)
                nc.scalar.activation(th[:, 0, 4:8], gt[0][:], AF.Tanh)
                if t > 0:
                    lp_exp(cpA, 0, pc)
                # DVE: ziA copy ; u2A/S2A ; tcn/h2 per feature ; dA/lsA
                if t + 1 < TR:
                    zi_copy(zpA, 0)
                u2_s2(0, pc, 0, th)
                tc_h2(0, pc, 0, th, tcn)
                u2_s2(0, pc, 1, th)
                tc_h2(0, pc, 1, th, tcn)
                if t > 0:
                    lp_dls(cpA, t, 0, t - 1, pc)
                # PE: coB(t-1) ; ig chunk B ; ziB
                if t > 0:
                    cpB = co_mm(t - 1, 1)
                gate_chunk(gt[1], 1, (0, 1, 2, 3))
                if t + 1 < TR:
                    zpB = zi_mm(t + 1, 1)
                # Act: tanh_igB ; expB(t-1)
                nc.scalar.activation(th[:, 1, 4:8], gt[1][:], AF.Tanh)
                if t > 0:
                    lp_exp(cpB, 1, pc)
                # DVE: ziB copy ; u2B/S2B ; tcn/h2 per feature ; dB/lsB
                if t + 1 < TR:
                    zi_copy(zpB, 1)
                u2_s2(1, pc, 0, th)
                tc_h2(1, pc, 0, th, tcn)
                u2_s2(1, pc, 1, th)
                tc_h2(1, pc, 1, th, tcn)
                if t > 0:
                    lp_dls(cpB, t, 1, t - 1, pc)
                # Pool: z, z^2, acc for both pairs
                if t > 0:
                    lp_pool(0, pc)
                    lp_pool(1, pc)

            # pre-reduce ls columns 0..T-2 (ready at loop end)
            rlp = spool.tile([H, BL, 1], F32, tag="rlp")
            nc.vector.tensor_reduce(rlp[:], lscols[:, :, 0:T - 1],
                                    axis=mybir.AxisListType.X, op=ALU.add)

            # drain: conv_out + logprob for t = TR-1
            for P in range(2):
                cp = co_mm(TR - 1, P)
                logprob(cp, TR, P, TR - 1, TR % 2)

            # ---- final reduction (ls cols 0:14 pre-reduced in-loop) ----
            rs = spool.tile([H, BL, 1], F32, tag="rs")
            rl = spool.tile([H, BL, 1], F32, tag="rl")
            for P in range(2):
                ps = slice(P * BP, (P + 1) * BP)
                nc.vector.tensor_reduce(rs[:, ps], acc[:, ps],
                                        axis=mybir.AxisListType.X, op=ALU.add)
                nc.vector.tensor_tensor(rl[:, ps], rlp[:, ps],
                                        lscols[:, ps, T - 1:T], op=ALU.add)
            comb = spool.tile([H, BL], F32, tag="comb")
            nc.vector.scalar_tensor_tensor(
                comb[:], rs[:, :, 0], -0.5, rl[:, :, 0], ALU.mult,
                ALU.subtract)
            fpt = zpool.tile([H, BP, W], F32, tag="zp")
            fps = fpt[0:BL, 0, 0:1]
            nc.tensor.matmul(fps, comb[:], ones[:], start=True, stop=True)
            osb = spool.tile([BL, 1], F32, tag="osb")
            nc.vector.tensor_copy(osb[:], fps)
            nc.sync.dma_start(out_d[:], osb[:])

    nc.compile()
    return nc


# module-level current index maps (captured by _build_program closures)
ZI_IDX = None
GATE_IDX = None
CO_IDX = None


def _get_program(key=0):
    global _CACHED
    if key not in _CACHED:
        _CACHED[key] = _build_program()
    return _CACHED[key]


def prepare(x, Win, b_in, Wih, b_ih, Whh, b_hh, Wout, b_out):
    """Build band data + program; returns (nc, in_maps, const)."""
    global ZI_IDX, GATE_IDX, CO_IDX
    x = np.asarray(x, np.float32)
    Win = np.asarray(Win, np.float32)
    Wih = np.asarray(Wih, np.float32)
    Whh = np.asarray(Whh, np.float32)
    Wout = np.asarray(Wout, np.float32)
    b_in = float(np.asarray(b_in, np.float32)[0])
    gb = np.asarray(b_ih, np.float32) + np.asarray(b_hh, np.float32)
    b0, b1 = [float(v) for v in np.asarray(b_out, np.float32)]

    pairs, zi_idx, gate_idx, co_idx = _build_band_pairs(
        Win, Wih, Whh, Wout, gb)
    ZI_IDX, GATE_IDX, CO_IDX = zi_idx, gate_idx, co_idx

    # band pair tiles -> [H, N*2*H] fp8 (lhsT layout: [K=H][pair][M=H])
    bp8 = np.ascontiguousarray(
        np.transpose(pairs, (2, 0, 1, 3))).astype(FP8).reshape(H, -1)

    sc = np.zeros((H, 8), np.float32)
    sc[:, 0] = np.exp(-b1)
    sc[:, 1] = -b0 * np.exp(-b1)
    sc[:, 2] = b_in
    sc[:, 3] = -b1
    sc[:, 4] = b0
    sc[:, 5] = 0.5

    in_maps = []
    for k in range(NCORES):
        xs = x[k * BL:(k + 1) * BL]                    # (BL, C, H, W)
        xt = np.transpose(xs, (1, 2, 0, 3))            # (C, H, BL, W)
        xpad = np.zeros((C, H, BL, WP), np.float32)
        xpad[:, :, :, 1:1 + W] = xt
        xq8 = np.ascontiguousarray(
            np.transpose(xpad, (1, 0, 2, 3))).astype(FP8).reshape(H, -1)
        xh16 = np.ascontiguousarray(
            np.transpose(xt, (1, 0, 2, 3))).astype(
                ml_dtypes.bfloat16).reshape(H, -1)
        in_maps.append({"xq": xq8, "xh": xh16, "bp": bp8, "sc": sc})

    nc = _get_program()
    global _last_in_maps
    _last_in_maps = in_maps
    const = -0.5 * LOG2PI * (H * W * C) - H * W * b1
    return nc, in_maps, const


def kernel(x, Win, b_in, Wih, b_ih, Whh, b_hh, Wout, b_out):
    nc, in_maps, const = prepare(x, Win, b_in, Wih, b_ih, Whh, b_hh,
                                 Wout, b_out)
    res = run_bass_kernel_spmd(nc, in_maps, core_ids=list(range(NCORES)))
    out = np.zeros((B,), np.float32)
    for k in range(NCORES):
        out[k * BL:(k + 1) * BL] = res.results[k]["out"].reshape(BL) + const
    return out

